# revision 1
# baseline (speedup 1.0000x reference)
"""Chunked sliding-window attention (B=2, T=8192, H=16, Dh=128, W=256) on 8
Trainium2 NeuronCores.

Sharding: 8 cores = 2 (batch) x 4 (head groups of 4 heads). Each core computes
q/k/v projections for its 512-wide slice of the 2048 projection dims, RoPE,
chunked attention for its 4 heads, and a partial output projection over its
512 rows of Wo^T. The host sums the 4 partial outputs per batch element.

Device layouts (host-prepared):
  xt   [128, 16, T]   x^T tiles: xt[p, kt, t] = x[b, t, kt*128+p]        (bf16)
  wq/wk[128, 16, 512] (Wq_perm)^T slice, rope-split row permutation      (bf16)
  wv   [128, 16, 512] Wv^T slice (unpermuted)                            (bf16)
  wo   [128, 4, 2048] Wo^T rows for this core's 512 dims                 (bf16)
  ccat [128, T]       [cos; cos] rope table (freq idx on partitions)     (bf16)
  scat [128, T]       [-sin; sin]                                        (bf16)
  mask [128, 2, 256]  transposed causal 0/1 masks for own-chunk kv tiles (bf16)

The rope row-permutation maps interleaved (re,im) pairs to split layout
(re block rows 0..63, im rows 64..127 per head); applied identically to q and
k it leaves scores invariant, and makes rope unit-stride on chip.

Attention is computed in transposed-score layout [kv, q]: softmax denominator
comes from an all-ones stationary matmul (broadcasts the per-q denominator
across all 128 partitions), masking is a 0/1 multiply after exp.
"""

import os

import numpy as np
import ml_dtypes

N_HEAD = 16
HEAD_DIM = 128
WINDOW = 256
THETA = 10000.0
B = 2
T = 8192
DM = 2048
KT = DM // 128      # 16 contraction tiles
HS = 4              # heads per core
DS = HS * HEAD_DIM  # 512 projection dims per core
BLK = 512           # tokens per pipeline block (2 chunks)
CH = WINDOW         # 256
SCALE = float(HEAD_DIM) ** -0.5

LAST_EXEC_NS = None
_NC = None

bf16 = ml_dtypes.bfloat16


def _build_nc(t_len=T):
    from contextlib import ExitStack

    import concourse.tile as tile
    from concourse import bacc, mybir

    fp32 = mybir.dt.float32
    b16 = mybir.dt.bfloat16

    nb = t_len // BLK
    nc = bacc.Bacc("TRN2", target_bir_lowering=False, debug=False)

    nblk = t_len // BLK
    xt = nc.dram_tensor(
        "xt", [nblk, 128, KT, BLK], b16, kind="ExternalInput"
    ).ap()
    wq = nc.dram_tensor("wq", [128, KT, DS], b16, kind="ExternalInput").ap()
    wk = nc.dram_tensor("wk", [128, KT, DS], b16, kind="ExternalInput").ap()
    wv = nc.dram_tensor("wv", [128, KT, DS], b16, kind="ExternalInput").ap()
    wo = nc.dram_tensor("wo", [128, HS, DM], b16, kind="ExternalInput").ap()
    ccat = nc.dram_tensor("ccat", [128, t_len], b16, kind="ExternalInput").ap()
    scat = nc.dram_tensor("scat", [128, t_len], b16, kind="ExternalInput").ap()
    mask = nc.dram_tensor("mask", [128, 2, CH], b16, kind="ExternalInput").ap()
    y = nc.dram_tensor("y", [t_len, DM], fp32, kind="ExternalOutput").ap()

    Exp = mybir.ActivationFunctionType.Exp

    with tile.TileContext(nc) as tc, ExitStack() as ctx:
        const = ctx.enter_context(tc.tile_pool(name="const", bufs=1))
        xt_p = ctx.enter_context(tc.tile_pool(name="xtp", bufs=2))
        raw_p = ctx.enter_context(tc.tile_pool(name="rawp", bufs=3))
        swp_p = ctx.enter_context(tc.tile_pool(name="swpp", bufs=3))
        tmp_p = ctx.enter_context(tc.tile_pool(name="tmpp", bufs=3))
        qr_p = ctx.enter_context(tc.tile_pool(name="qrp", bufs=8))
        kr_p = ctx.enter_context(tc.tile_pool(name="krp", bufs=10))
        v_p = ctx.enter_context(tc.tile_pool(name="vp", bufs=10))
        e_p = ctx.enter_context(tc.tile_pool(name="ep", bufs=12))
        rc_p = ctx.enter_context(tc.tile_pool(name="rcp", bufs=4))
        ot_p = ctx.enter_context(tc.tile_pool(name="otp", bufs=24))
        y_p = ctx.enter_context(tc.tile_pool(name="yp", bufs=2))
        tab_p = ctx.enter_context(tc.tile_pool(name="tabp", bufs=3))
        ps_big = ctx.enter_context(tc.tile_pool(name="psbig", bufs=2, space="PSUM"))
        ps_st = ctx.enter_context(tc.tile_pool(name="psst", bufs=3, space="PSUM"))
        ps_do = ctx.enter_context(tc.tile_pool(name="psdo", bufs=3, space="PSUM"))

        # Const loads are split into pieces and emitted in the order the first
        # pipeline block consumes them, so the first matmul isn't gated on the
        # full 12 MB of constants.
        wq_sb = const.tile([128, KT, DS], b16)
        wk_sb = const.tile([128, KT, DS], b16)
        wv_sb = const.tile([128, KT, DS], b16)
        wo_sb = const.tile([128, HS, DM], b16)
        mask_sb = const.tile([128, 2, CH], b16)
        ones_sb = const.tile([128, 128], b16)
        nc.vector.memset(ones_sb, 1.0)
        # Block 0's x tiles prefetched here so the first projection matmuls
        # only wait on ~4 MB, not the full constant set.
        def fetch_tables(t0):
            cc = tab_p.tile([128, BLK], b16, tag="cc")
            nc.sync.dma_start(cc, ccat[:, t0 : t0 + BLK])
            sc = tab_p.tile([128, BLK], b16, tag="sc")
            nc.sync.dma_start(sc, scat[:, t0 : t0 + BLK])
            return cc, sc

        xt_first = xt_p.tile([128, KT, BLK], b16, tag="xt")
        nc.sync.dma_start(mask_sb, mask)
        nc.sync.dma_start(wq_sb[:, 0:4, :], wq[:, 0:4, :])
        nc.sync.dma_start(xt_first[:, 0:4, :], xt[0][:, 0:4, :])
        nc.sync.dma_start(wq_sb[:, 4:KT, :], wq[:, 4:KT, :])
        nc.sync.dma_start(xt_first[:, 4:KT, :], xt[0][:, 4:KT, :])
        tab_first = fetch_tables(0)
        nc.sync.dma_start(wk_sb[:, 0:8, :], wk[:, 0:8, :])
        nc.sync.dma_start(wk_sb[:, 8:KT, :], wk[:, 8:KT, :])
        nc.sync.dma_start(wv_sb[:, 0:8, :], wv[:, 0:8, :])
        nc.sync.dma_start(wv_sb[:, 8:KT, :], wv[:, 8:KT, :])
        tab_second = fetch_tables(BLK) if nb > 1 else None
        xt_second = None
        if nb > 1:
            xt_second = xt_p.tile([128, KT, BLK], b16, tag="xt")
            nc.sync.dma_start(xt_second, xt[1])
        for h in range(HS):
            nc.sync.dma_start(wo_sb[:, h, :], wo[:, h, :])

        prev_k = [None] * HS
        prev_v = [None, None]
        pend_ot = None
        for blk in range(nb):
            t0 = blk * BLK
            if blk == 0:
                xt_sb = xt_first
            elif blk == 1 and xt_second is not None:
                xt_sb = xt_second
            else:
                xt_sb = xt_p.tile([128, KT, BLK], b16, tag="xt")
                nc.sync.dma_start(xt_sb, xt[blk])

            if blk == 0:
                c_sl, s_sl = tab_first
            elif blk == 1 and tab_second is not None:
                c_sl, s_sl = tab_second
            else:
                c_sl, s_sl = fetch_tables(t0)
            cur_q = []
            cur_k = []
            for h in range(HS):
                for w_sb, dst in ((wq_sb, cur_q), (wk_sb, cur_k)):
                    ps = ps_big.tile([128, BLK], fp32, tag="psbig")
                    for k in range(KT):
                        nc.tensor.matmul(
                            ps,
                            lhsT=w_sb[:, k, h * 128 : (h + 1) * 128],
                            rhs=xt_sb[:, k, :],
                            start=(k == 0),
                            stop=(k == KT - 1),
                        )
                    raw = raw_p.tile([128, BLK], b16, tag="raw")
                    nc.scalar.copy(raw, ps)
                    # swap the (re, im) halves via SBUF->SBUF DMA (DVE lanes
                    # cannot cross partitions)
                    swp = swp_p.tile([128, BLK], b16, tag="swp")
                    nc.sync.dma_start(swp[0:64, :], raw[64:128, :])
                    nc.sync.dma_start(swp[64:128, :], raw[0:64, :])
                    t1 = tmp_p.tile([128, BLK], b16, tag="t1")
                    nc.vector.tensor_mul(t1, raw, c_sl)
                    t2 = tmp_p.tile([128, BLK], b16, tag="t2")
                    nc.vector.tensor_mul(t2, swp, s_sl)
                    if dst is cur_q:
                        rot = qr_p.tile([128, BLK], b16, tag="qr")
                    else:
                        rot = kr_p.tile([128, BLK], b16, tag="kr")
                    nc.vector.tensor_add(rot, t1, t2)
                    dst.append(rot)

            cur_v = []
            for tt in range(4):
                ps = ps_big.tile([128, BLK], fp32, tag="psbig")
                for k in range(KT):
                    nc.tensor.matmul(
                        ps,
                        lhsT=xt_sb[:, k, tt * 128 : (tt + 1) * 128],
                        rhs=wv_sb[:, k, :],
                        start=(k == 0),
                        stop=(k == KT - 1),
                    )
                vt = v_p.tile([128, DS], b16, tag="v")
                nc.vector.tensor_copy(out=vt, in_=ps)
                cur_v.append(vt)

            ot_tiles = {}
            for ci in range(2):
                c = 2 * blk + ci
                qoff = ci * CH
                js = [2, 3] if c == 0 else [0, 1, 2, 3]
                for h in range(HS):
                    q_sl = cur_q[h][:, qoff : qoff + CH]
                    es = []
                    for j in js:
                        if j < 2:
                            if ci == 1:
                                ksrc = cur_k[h][:, j * 128 : (j + 1) * 128]
                            else:
                                ksrc = prev_k[h][:, CH + j * 128 : CH + (j + 1) * 128]
                        else:
                            ksrc = cur_k[h][:, qoff + (j - 2) * 128 : qoff + (j - 1) * 128]
                        st = ps_st.tile([128, CH], fp32, tag="st")
                        nc.tensor.matmul(st, lhsT=ksrc, rhs=q_sl, start=True, stop=True)
                        e = e_p.tile([128, CH], b16, tag="e")
                        nc.scalar.activation(e, st, Exp, scale=SCALE)
                        if j >= 2:
                            nc.vector.tensor_mul(e, e, mask_sb[:, j - 2, :])
                        es.append((j, e))
                    dn = ps_do.tile([128, CH], fp32, tag="do")
                    for i, (j, e) in enumerate(es):
                        nc.tensor.matmul(
                            dn, lhsT=ones_sb, rhs=e,
                            start=(i == 0), stop=(i == len(es) - 1),
                        )
                    ou = ps_do.tile([128, CH], fp32, tag="do")
                    for i, (j, e) in enumerate(es):
                        if j < 2:
                            vsrc = cur_v[j] if ci == 1 else prev_v[j]
                        else:
                            vsrc = cur_v[2 * ci + (j - 2)]
                        nc.tensor.matmul(
                            ou, lhsT=vsrc[:, h * 128 : (h + 1) * 128], rhs=e,
                            start=(i == 0), stop=(i == len(es) - 1),
                        )
                    rc = rc_p.tile([128, CH], fp32, tag="rc")
                    nc.vector.reciprocal_approx_fast(out=rc, in_=dn)
                    ot = ot_p.tile([128, CH], b16, tag="ot")
                    nc.vector.tensor_mul(ot, ou, rc)
                    ot_tiles[(h, ci)] = ot

            def emit_oproj(ot_map, base_t0):
                for tt in range(4):
                    ci, sub = tt // 2, tt % 2
                    ysb = y_p.tile([128, DM], fp32, tag="y")
                    for ct in range(4):
                        yps = ps_big.tile([128, 512], fp32, tag="psbig")
                        for h in range(HS):
                            nc.tensor.matmul(
                                yps,
                                lhsT=ot_map[(h, ci)][:, sub * 128 : (sub + 1) * 128],
                                rhs=wo_sb[:, h, ct * 512 : (ct + 1) * 512],
                                start=(h == 0),
                                stop=(h == HS - 1),
                            )
                        if ct % 2 == 0:
                            nc.scalar.copy(ysb[:, ct * 512 : (ct + 1) * 512], yps)
                        else:
                            nc.vector.tensor_copy(
                                out=ysb[:, ct * 512 : (ct + 1) * 512], in_=yps
                            )
                    nc.sync.dma_start(
                        y[base_t0 + tt * 128 : base_t0 + (tt + 1) * 128, :], ysb
                    )

            # o-projection deferred one block so it never heads the PE queue
            # while wo / next xt are still in flight
            if pend_ot is not None:
                emit_oproj(pend_ot[0], pend_ot[1])
            pend_ot = (ot_tiles, t0)
            if blk == nb - 1:
                emit_oproj(ot_tiles, t0)
                pend_ot = None

            prev_k = cur_k
            prev_v = cur_v[2:4]

    nc.compile()
    return nc


def _rope_perm():
    perm = np.empty(DM, np.int64)
    for h in range(N_HEAD):
        base = h * HEAD_DIM
        perm[base : base + 64] = base + 2 * np.arange(64)
        perm[base + 64 : base + 128] = base + 2 * np.arange(64) + 1
    return perm


def _prep_inputs(x, Wq, Wk, Wv, Wo, t_len=T):
    """Build per-core in_maps. Cores 0-3: batch 0, head groups 0-3; 4-7: batch 1."""
    x = np.asarray(x, dtype=np.float32)
    Wq = np.asarray(Wq, dtype=np.float32)
    Wk = np.asarray(Wk, dtype=np.float32)
    Wv = np.asarray(Wv, dtype=np.float32)
    Wo = np.asarray(Wo, dtype=np.float32)
    nb_b = x.shape[0]

    perm = _rope_perm()
    wqT = np.ascontiguousarray(Wq[perm].T).astype(bf16)  # [K, dout_perm]
    wkT = np.ascontiguousarray(Wk[perm].T).astype(bf16)
    wvT = np.ascontiguousarray(Wv.T).astype(bf16)
    woT = np.ascontiguousarray(Wo.T).astype(bf16)        # [d, c]

    # xt[blk, p, kt, t_in_blk] = x[b, blk*BLK + t, kt*128+p] — block-major so
    # each block's slab is one fully-contiguous DMA read per partition
    nblk = t_len // BLK
    xts = []
    for b in range(nb_b):
        xT = x[b].T.reshape(KT, 128, nblk, BLK)
        xts.append(np.ascontiguousarray(xT.transpose(2, 1, 0, 3)).astype(bf16))

    wq_s, wk_s, wv_s, wo_s = [], [], [], []
    for hg in range(4):
        sl = slice(hg * DS, (hg + 1) * DS)
        wq_s.append(np.ascontiguousarray(
            wqT[:, sl].reshape(KT, 128, DS).transpose(1, 0, 2)).astype(bf16))
        wk_s.append(np.ascontiguousarray(
            wkT[:, sl].reshape(KT, 128, DS).transpose(1, 0, 2)).astype(bf16))
        wv_s.append(np.ascontiguousarray(
            wvT[:, sl].reshape(KT, 128, DS).transpose(1, 0, 2)).astype(bf16))
        wo_s.append(np.ascontiguousarray(
            woT[sl].reshape(HS, 128, DM).transpose(1, 0, 2)).astype(bf16))

    inv = 1.0 / THETA ** (np.arange(0, HEAD_DIM, 2, dtype=np.float32) / HEAD_DIM)
    fr = np.outer(inv, np.arange(t_len, dtype=np.float32))  # [64, T]
    cosT = np.cos(fr).astype(np.float32)
    sinT = np.sin(fr).astype(np.float32)
    ccat = np.concatenate([cosT, cosT], axis=0).astype(bf16)   # [128, T]
    scat = np.concatenate([-sinT, sinT], axis=0).astype(bf16)  # [128, T]

    r = np.arange(128)[:, None]
    qc = np.arange(CH)[None, :]
    mask = np.stack([(r <= qc), (128 + r <= qc)], axis=1).astype(bf16)  # [128,2,256]

    in_maps = []
    for core in range(8):
        b, hg = core // 4, core % 4
        in_maps.append({
            "xt": xts[b], "wq": wq_s[hg], "wk": wk_s[hg], "wv": wv_s[hg],
            "wo": wo_s[hg], "ccat": ccat, "scat": scat, "mask": mask,
        })
    return in_maps


def kernel(x, Wq, Wk, Wv, Wo):
    global _NC, LAST_EXEC_NS
    from concourse.bass_utils import run_bass_kernel_spmd

    profile = bool(os.environ.get("KERNEL_PROFILE"))
    if profile:
        try:
            import hook_util
            hook_util.install()
            hook_util.patch_upload()
        except ImportError:
            profile = False

    in_maps = _prep_inputs(x, Wq, Wk, Wv, Wo)
    if _NC is None:
        _NC = _build_nc()

    kwargs = {}
    if profile:
        kwargs["tmpdir"] = os.environ.get("KERNEL_TRACE_DIR") or None
    res = run_bass_kernel_spmd(
        _NC, in_maps, core_ids=list(range(8)), trace=profile, **kwargs
    )
    LAST_EXEC_NS = res.exec_time_ns

    out = np.zeros((B, T, DM), dtype=np.float32)
    for core in range(8):
        out[core // 4] += res.results[core]["y"]
    return out



# revision 3
# speedup vs baseline: 1.0703x; 1.0703x over previous
"""Chunked sliding-window attention (B=2, T=8192, H=16, Dh=128, W=256) on 8
Trainium2 NeuronCores.

Sharding: 8 cores = 2 (batch) x 4 (head groups of 4 heads). Each core computes
q/k/v projections for its 512-wide slice of the 2048 projection dims, RoPE,
chunked attention for its 4 heads, and a partial output projection over its
512 rows of Wo^T. The host sums the 4 partial outputs per batch element.

Device layouts (host-prepared):
  xt   [128, 16, T]   x^T tiles: xt[p, kt, t] = x[b, t, kt*128+p]        (bf16)
  wq/wk[128, 16, 512] (Wq_perm)^T slice, rope-split row permutation      (bf16)
  wv   [128, 16, 512] Wv^T slice (unpermuted)                            (bf16)
  wo   [128, 4, 2048] Wo^T rows for this core's 512 dims                 (bf16)
  ccat [128, T]       [cos; cos] rope table (freq idx on partitions)     (bf16)
  scat [128, T]       [-sin; sin]                                        (bf16)
  mask [128, 384]     causal masks: cols 0:256 own-chunk kv tile 0,
                      cols 256:384 own-chunk kv tile 1 vs q-half b       (bf16)

Attention is computed in transposed-score layout [kv, q].  Per (head, chunk):
scores for the 4 kv tiles land pairwise in two [128,512] PSUM banks (the last
own-chunk kv tile only against the upper q-half - the lower half is fully
causal-masked), one exp per pair, one fused mask multiply, then the softmax
denominator comes from a single all-ones stationary matmul over the
DVE/GpSimd-summed exp tiles.  Output projection chains for the previous block
are interleaved into the attention units as PE filler so the engine never
waits on the exp->mask->sum chain.
"""

import os
from collections import deque

import numpy as np
import ml_dtypes

N_HEAD = 16
HEAD_DIM = 128
WINDOW = 256
THETA = 10000.0
B = 2
T = 8192
DM = 2048
KT = DM // 128      # 16 contraction tiles
HS = 4              # heads per core
DS = HS * HEAD_DIM  # 512 projection dims per core
BLK = 512           # tokens per pipeline block (2 chunks)
CH = WINDOW         # 256
SCALE = float(HEAD_DIM) ** -0.5

LAST_EXEC_NS = None
_NC = None

bf16 = ml_dtypes.bfloat16


def _build_nc(t_len=T):
    from contextlib import ExitStack

    import concourse.tile as tile
    from concourse import bacc, mybir

    fp32 = mybir.dt.float32
    b16 = mybir.dt.bfloat16

    nc = bacc.Bacc("TRN2", target_bir_lowering=False, debug=False)

    nb = t_len // BLK
    xt = nc.dram_tensor(
        "xt", [nb, 128, KT, BLK], b16, kind="ExternalInput"
    ).ap()
    wq = nc.dram_tensor("wq", [128, KT, DS], b16, kind="ExternalInput").ap()
    wk = nc.dram_tensor("wk", [128, KT, DS], b16, kind="ExternalInput").ap()
    wv = nc.dram_tensor("wv", [128, KT, DS], b16, kind="ExternalInput").ap()
    wo = nc.dram_tensor("wo", [128, HS, DM], b16, kind="ExternalInput").ap()
    ccat = nc.dram_tensor("ccat", [128, t_len], b16, kind="ExternalInput").ap()
    scat = nc.dram_tensor("scat", [128, t_len], b16, kind="ExternalInput").ap()
    mask = nc.dram_tensor("mask", [128, 384], b16, kind="ExternalInput").ap()
    y = nc.dram_tensor("y", [t_len, DM], fp32, kind="ExternalOutput").ap()

    Exp = mybir.ActivationFunctionType.Exp

    with tile.TileContext(nc) as tc, ExitStack() as ctx:
        const = ctx.enter_context(tc.tile_pool(name="const", bufs=1))
        xt_p = ctx.enter_context(tc.tile_pool(name="xtp", bufs=2))
        raw_p = ctx.enter_context(tc.tile_pool(name="rawp", bufs=3))
        swp_p = ctx.enter_context(tc.tile_pool(name="swpp", bufs=3))
        tmp_p = ctx.enter_context(tc.tile_pool(name="tmpp", bufs=3))
        qr_p = ctx.enter_context(tc.tile_pool(name="qrp", bufs=8))
        kr_p = ctx.enter_context(tc.tile_pool(name="krp", bufs=10))
        v_p = ctx.enter_context(tc.tile_pool(name="vp", bufs=10))
        e01_p = ctx.enter_context(tc.tile_pool(name="e01p", bufs=5))
        e23_p = ctx.enter_context(tc.tile_pool(name="e23p", bufs=5))
        es_p = ctx.enter_context(tc.tile_pool(name="esp", bufs=4))
        rc_p = ctx.enter_context(tc.tile_pool(name="rcp", bufs=4))
        ot_p = ctx.enter_context(tc.tile_pool(name="otp", bufs=20))
        y_p = ctx.enter_context(tc.tile_pool(name="yp", bufs=3))
        tab_p = ctx.enter_context(tc.tile_pool(name="tabp", bufs=3))
        # PSUM: 8 banks of [128,512]f32.  3 shared by proj+oproj chains,
        # 3 for the score pairs, 2 for the (AV out | denominator) pairs.
        ps_big = ctx.enter_context(tc.tile_pool(name="psbig", bufs=3, space="PSUM"))
        ps_sc = ctx.enter_context(tc.tile_pool(name="pssc", bufs=3, space="PSUM"))
        ps_od = ctx.enter_context(tc.tile_pool(name="psod", bufs=2, space="PSUM"))

        wq_sb = const.tile([128, KT, DS], b16)
        wk_sb = const.tile([128, KT, DS], b16)
        wv_sb = const.tile([128, KT, DS], b16)
        wo_sb = const.tile([128, HS, DM], b16)
        mask_sb = const.tile([128, 384], b16)
        ones_sb = const.tile([128, 128], b16)
        nc.vector.memset(ones_sb, 1.0)

        def fetch_tables(t0):
            cc = tab_p.tile([128, BLK], b16, tag="cc")
            nc.sync.dma_start(cc, ccat[:, t0 : t0 + BLK])
            sc = tab_p.tile([128, BLK], b16, tag="sc")
            nc.sync.dma_start(sc, scat[:, t0 : t0 + BLK])
            return cc, sc

        # Const loads interleaved so the first projection chain starts after
        # ~0.5 MB instead of the full constant set.
        xt_first = xt_p.tile([128, KT, BLK], b16, tag="xt")
        nc.sync.dma_start(mask_sb, mask)
        nc.sync.dma_start(wq_sb[:, 0:2, :], wq[:, 0:2, :])
        nc.sync.dma_start(xt_first[:, 0:2, :], xt[0][:, 0:2, :])
        nc.sync.dma_start(wq_sb[:, 2:4, :], wq[:, 2:4, :])
        nc.sync.dma_start(xt_first[:, 2:4, :], xt[0][:, 2:4, :])
        tab_first = fetch_tables(0)
        nc.sync.dma_start(wq_sb[:, 4:8, :], wq[:, 4:8, :])
        nc.sync.dma_start(xt_first[:, 4:8, :], xt[0][:, 4:8, :])
        nc.sync.dma_start(wq_sb[:, 8:KT, :], wq[:, 8:KT, :])
        nc.sync.dma_start(xt_first[:, 8:KT, :], xt[0][:, 8:KT, :])
        nc.sync.dma_start(wk_sb[:, 0:4, :], wk[:, 0:4, :])
        nc.sync.dma_start(wk_sb[:, 4:8, :], wk[:, 4:8, :])
        nc.sync.dma_start(wk_sb[:, 8:KT, :], wk[:, 8:KT, :])
        nc.sync.dma_start(wv_sb[:, 0:8, :], wv[:, 0:8, :])
        nc.sync.dma_start(wv_sb[:, 8:KT, :], wv[:, 8:KT, :])
        tab_second = fetch_tables(BLK) if nb > 1 else None
        xt_second = None
        if nb > 1:
            xt_second = xt_p.tile([128, KT, BLK], b16, tag="xt")
            nc.sync.dma_start(xt_second, xt[1])
        for h in range(HS):
            nc.sync.dma_start(wo_sb[:, h, :], wo[:, h, :])

        def make_proj(xt_sb, c_sl, s_sl, qk_first=False):
            """Returns (thunks, cur_q, cur_k, cur_v); each thunk emits one
            PE chain (+rope / cast tail)."""
            cur_q = [None] * HS
            cur_k = [None] * HS
            cur_v = [None] * 4

            def qk_chain(h, w_sb, dst, pool, tag):
                def t():
                    ps = ps_big.tile([128, BLK], fp32, tag="psbig")
                    for k in range(KT):
                        nc.tensor.matmul(
                            ps,
                            lhsT=w_sb[:, k, h * 128 : (h + 1) * 128],
                            rhs=xt_sb[:, k, :],
                            start=(k == 0),
                            stop=(k == KT - 1),
                        )
                    raw = raw_p.tile([128, BLK], b16, tag="raw")
                    nc.scalar.copy(raw, ps)
                    # swap (re, im) halves via SBUF->SBUF DMA (DVE lanes
                    # cannot cross partitions)
                    swp = swp_p.tile([128, BLK], b16, tag="swp")
                    nc.sync.dma_start(swp[0:64, :], raw[64:128, :])
                    nc.sync.dma_start(swp[64:128, :], raw[0:64, :])
                    t1 = tmp_p.tile([128, BLK], b16, tag="t1")
                    nc.vector.tensor_mul(t1, raw, c_sl)
                    t2 = tmp_p.tile([128, BLK], b16, tag="t2")
                    nc.vector.tensor_mul(t2, swp, s_sl)
                    rot = pool.tile([128, BLK], b16, tag=tag)
                    nc.vector.tensor_add(rot, t1, t2)
                    dst[h] = rot
                return t

            def v_chain(tt):
                def t():
                    ps = ps_big.tile([128, BLK], fp32, tag="psbig")
                    for k in range(KT):
                        nc.tensor.matmul(
                            ps,
                            lhsT=xt_sb[:, k, tt * 128 : (tt + 1) * 128],
                            rhs=wv_sb[:, k, :],
                            start=(k == 0),
                            stop=(k == KT - 1),
                        )
                    vt = v_p.tile([128, DS], b16, tag="v")
                    nc.vector.tensor_copy(out=vt, in_=ps)
                    cur_v[tt] = vt
                return t

            thunks = []
            if qk_first:
                # block 0: wk arrives after wq+xt, so run all q chains first
                for h in range(HS):
                    thunks.append(qk_chain(h, wq_sb, cur_q, qr_p, "qr"))
                for h in range(HS):
                    thunks.append(qk_chain(h, wk_sb, cur_k, kr_p, "kr"))
            else:
                for h in range(HS):
                    thunks.append(qk_chain(h, wq_sb, cur_q, qr_p, "qr"))
                    thunks.append(qk_chain(h, wk_sb, cur_k, kr_p, "kr"))
            for tt in range(4):
                thunks.append(v_chain(tt))
            return thunks, cur_q, cur_k, cur_v

        def make_oproj_thunks(ot_map, base_t0):
            """16 thunks: one (tt, ct) oproj chain each; DMA per tt tile."""
            ysbs = {}
            thunks = []
            for tt in range(4):
                ci, sub = tt // 2, tt % 2
                for ct in range(4):
                    def t(tt=tt, ct=ct, ci=ci, sub=sub):
                        if ct == 0:
                            ysbs[tt] = y_p.tile(
                                [128, DM], fp32, tag="y", name="ysb"
                            )
                        ysb = ysbs[tt]
                        yps = ps_big.tile([128, 512], fp32, tag="psbig")
                        for h in range(HS):
                            nc.tensor.matmul(
                                yps,
                                lhsT=ot_map[(h, ci)][:, sub * 128 : (sub + 1) * 128],
                                rhs=wo_sb[:, h, ct * 512 : (ct + 1) * 512],
                                start=(h == 0),
                                stop=(h == HS - 1),
                            )
                        if ct % 2 == 0:
                            nc.scalar.copy(ysb[:, ct * 512 : (ct + 1) * 512], yps)
                        else:
                            nc.vector.tensor_copy(
                                out=ysb[:, ct * 512 : (ct + 1) * 512], in_=yps
                            )
                        if ct == 3:
                            nc.sync.dma_start(
                                y[base_t0 + tt * 128 : base_t0 + (tt + 1) * 128, :],
                                ysb,
                            )
                    thunks.append(t)
            return thunks

        class Unit:
            """One (chunk-half, head) attention unit."""

            def __init__(self, blk, ci, h, cur_q, cur_k, cur_v, prev_k, prev_v):
                self.ci, self.h = ci, h
                self.c = 2 * blk + ci
                self.qoff = ci * CH
                self.full = self.c != 0
                self.cur_q, self.cur_k, self.cur_v = cur_q, cur_k, cur_v
                self.prev_k, self.prev_v = prev_k, prev_v

            def ksrc(self, j):
                h, ci, qoff = self.h, self.ci, self.qoff
                if j < 2:
                    if ci == 1:
                        return self.cur_k[h][:, j * 128 : (j + 1) * 128]
                    return self.prev_k[h][:, CH + j * 128 : CH + (j + 1) * 128]
                return self.cur_k[h][:, qoff + (j - 2) * 128 : qoff + (j - 1) * 128]

            def vsrc(self, j):
                if j < 2:
                    vt = self.cur_v[j] if self.ci == 1 else self.prev_v[j]
                else:
                    vt = self.cur_v[2 * self.ci + (j - 2)]
                return vt[:, self.h * 128 : (self.h + 1) * 128]

            def emit_scores(self):
                q_sl = self.cur_q[self.h][:, self.qoff : self.qoff + CH]
                if self.full:
                    st01 = ps_sc.tile([128, 512], fp32, tag="sc")
                    for j in (0, 1):
                        nc.tensor.matmul(
                            st01[:, j * CH : (j + 1) * CH],
                            lhsT=self.ksrc(j), rhs=q_sl,
                            start=True, stop=True, skip_group_check=True,
                        )
                    e01 = e01_p.tile([128, 512], b16, tag="e01")
                    nc.scalar.activation(e01, st01, Exp, scale=SCALE)
                    self.e01 = e01
                st23 = ps_sc.tile([128, 512], fp32, tag="sc")
                nc.tensor.matmul(
                    st23[:, 0:CH], lhsT=self.ksrc(2), rhs=q_sl,
                    start=True, stop=True, skip_group_check=True,
                )
                # last own-chunk kv tile: lower q-half fully causal-masked
                nc.tensor.matmul(
                    st23[:, CH : CH + 128], lhsT=self.ksrc(3),
                    rhs=q_sl[:, 128:CH],
                    start=True, stop=True, skip_group_check=True,
                )
                e23 = e23_p.tile([128, 384], b16, tag="e23")
                nc.scalar.activation(e23, st23[:, 0:384], Exp, scale=SCALE)
                self.e23 = e23
                if self.full:
                    es = es_p.tile([128, CH], b16, tag="es")
                    nc.vector.tensor_add(es, e01[:, 0:CH], e01[:, CH:512])
                    self.es = es
                else:
                    self.es = None
                nc.vector.tensor_mul(e23, e23, mask_sb)
                if self.full:
                    nc.gpsimd.tensor_add(self.es, self.es, e23[:, 0:CH])
                    nc.gpsimd.tensor_add(
                        self.es[:, 128:CH], self.es[:, 128:CH], e23[:, CH:384]
                    )

            def emit_avdn(self, ot_map):
                od = ps_od.tile([128, 512], fp32, tag="od")
                ou = od[:, 0:CH]
                dn = od[:, CH:512]
                if self.full:
                    nc.tensor.matmul(
                        ou, lhsT=self.vsrc(0), rhs=self.e01[:, 0:CH],
                        start=True, stop=False, skip_group_check=True,
                    )
                    nc.tensor.matmul(
                        ou, lhsT=self.vsrc(1), rhs=self.e01[:, CH:512],
                        start=False, stop=False, skip_group_check=True,
                    )
                    nc.tensor.matmul(
                        ou, lhsT=self.vsrc(2), rhs=self.e23[:, 0:CH],
                        start=False, stop=False, skip_group_check=True,
                    )
                else:
                    nc.tensor.matmul(
                        ou, lhsT=self.vsrc(2), rhs=self.e23[:, 0:CH],
                        start=True, stop=False, skip_group_check=True,
                    )
                nc.tensor.matmul(
                    ou[:, 128:CH], lhsT=self.vsrc(3), rhs=self.e23[:, CH:384],
                    start=False, stop=True, skip_group_check=True,
                )
                if self.es is not None:
                    nc.tensor.matmul(
                        dn, lhsT=ones_sb, rhs=self.es,
                        start=True, stop=True, skip_group_check=True,
                    )
                else:
                    nc.tensor.matmul(
                        dn, lhsT=ones_sb, rhs=self.e23[:, 0:CH],
                        start=True, stop=False, skip_group_check=True,
                    )
                    nc.tensor.matmul(
                        dn[:, 128:CH], lhsT=ones_sb, rhs=self.e23[:, CH:384],
                        start=False, stop=True, skip_group_check=True,
                    )
                rc = rc_p.tile([128, CH], fp32, tag="rc")
                nc.vector.reciprocal_approx_fast(out=rc, in_=dn)
                ot = ot_p.tile([128, CH], b16, tag="ot")
                nc.vector.tensor_mul(ot, ou, rc)
                ot_map[(self.h, self.ci)] = ot

        prev_k = [None] * HS
        prev_v = [None, None]
        pend = None  # (ot_map, t0) of previous block
        pending_proj = None  # blk1's projections, emitted during blk0's att

        for blk in range(nb):
            t0 = blk * BLK
            if blk == 0:
                xt_sb, (c_sl, s_sl) = xt_first, tab_first
            elif blk == 1:
                xt_sb, (c_sl, s_sl) = xt_second, tab_second
            else:
                xt_sb = xt_p.tile([128, KT, BLK], b16, tag="xt")
                nc.sync.dma_start(xt_sb, xt[blk])
                c_sl, s_sl = fetch_tables(t0)

            if pending_proj is not None:
                cur_q, cur_k, cur_v = pending_proj
                pending_proj = None
            else:
                thunks, cur_q, cur_k, cur_v = make_proj(
                    xt_sb, c_sl, s_sl, qk_first=(blk == 0)
                )
                for t in thunks:
                    t()

            fillers = deque()
            if pend is not None:
                fillers.extend(make_oproj_thunks(*pend))
            if blk == 0 and nb > 1:
                thunks1, cq1, ck1, cv1 = make_proj(xt_second, *tab_second)
                fillers.extend(thunks1)
                pending_proj = (cq1, ck1, cv1)

            last = blk == nb - 1
            quota = 3 if last else 2
            ot_map = {}
            own = make_oproj_thunks(ot_map, t0) if last else None

            units = [
                Unit(blk, ci, h, cur_q, cur_k, cur_v, prev_k, prev_v)
                for ci in range(2)
                for h in range(HS)
            ]
            prev_u = None
            for ui, u in enumerate(units):
                u.emit_scores()
                for _ in range(quota):
                    if fillers:
                        fillers.popleft()()
                if prev_u is not None:
                    prev_u.emit_avdn(ot_map)
                if last and ui == 4:
                    # ci0's ot tiles complete after the ui==4 emit_avdn;
                    # its oproj chains become eligible filler
                    fillers.extend(own[:8])
                prev_u = u
            units[-1].emit_avdn(ot_map)
            while fillers:
                fillers.popleft()()
            if last:
                for t in own[8:]:
                    t()
            else:
                pend = (ot_map, t0)

            prev_k = cur_k
            prev_v = cur_v[2:4]

    nc.compile()
    return nc


def _rope_perm():
    perm = np.empty(DM, np.int64)
    for h in range(N_HEAD):
        base = h * HEAD_DIM
        perm[base : base + 64] = base + 2 * np.arange(64)
        perm[base + 64 : base + 128] = base + 2 * np.arange(64) + 1
    return perm


def _prep_inputs(x, Wq, Wk, Wv, Wo, t_len=T):
    """Build per-core in_maps. Cores 0-3: batch 0, head groups 0-3; 4-7: batch 1."""
    x = np.asarray(x, dtype=np.float32)
    Wq = np.asarray(Wq, dtype=np.float32)
    Wk = np.asarray(Wk, dtype=np.float32)
    Wv = np.asarray(Wv, dtype=np.float32)
    Wo = np.asarray(Wo, dtype=np.float32)
    nb_b = x.shape[0]

    perm = _rope_perm()
    wqT = np.ascontiguousarray(Wq[perm].T).astype(bf16)  # [K, dout_perm]
    wkT = np.ascontiguousarray(Wk[perm].T).astype(bf16)
    wvT = np.ascontiguousarray(Wv.T).astype(bf16)
    woT = np.ascontiguousarray(Wo.T).astype(bf16)        # [d, c]

    # xt[blk, p, kt, t_in_blk] = x[b, blk*BLK + t, kt*128+p] — block-major so
    # each block's slab is one fully-contiguous DMA read per partition
    nblk = t_len // BLK
    xts = []
    for b in range(nb_b):
        xT = x[b].T.reshape(KT, 128, nblk, BLK)
        xts.append(np.ascontiguousarray(xT.transpose(2, 1, 0, 3)).astype(bf16))

    wq_s, wk_s, wv_s, wo_s = [], [], [], []
    for hg in range(4):
        sl = slice(hg * DS, (hg + 1) * DS)
        wq_s.append(np.ascontiguousarray(
            wqT[:, sl].reshape(KT, 128, DS).transpose(1, 0, 2)).astype(bf16))
        wk_s.append(np.ascontiguousarray(
            wkT[:, sl].reshape(KT, 128, DS).transpose(1, 0, 2)).astype(bf16))
        wv_s.append(np.ascontiguousarray(
            wvT[:, sl].reshape(KT, 128, DS).transpose(1, 0, 2)).astype(bf16))
        wo_s.append(np.ascontiguousarray(
            woT[sl].reshape(HS, 128, DM).transpose(1, 0, 2)).astype(bf16))

    inv = 1.0 / THETA ** (np.arange(0, HEAD_DIM, 2, dtype=np.float32) / HEAD_DIM)
    fr = np.outer(inv, np.arange(t_len, dtype=np.float32))  # [64, T]
    cosT = np.cos(fr).astype(np.float32)
    sinT = np.sin(fr).astype(np.float32)
    ccat = np.concatenate([cosT, cosT], axis=0).astype(bf16)   # [128, T]
    scat = np.concatenate([-sinT, sinT], axis=0).astype(bf16)  # [128, T]

    r = np.arange(128)[:, None]
    qc = np.arange(CH)[None, :]
    qh = np.arange(128)[None, :]
    mask = np.concatenate(
        [(r <= qc), (r <= qh)], axis=1
    ).astype(bf16)  # [128, 384]: own kv tile 0 vs all q; own kv tile 1 vs q-half b

    in_maps = []
    for core in range(8):
        b, hg = core // 4, core % 4
        in_maps.append({
            "xt": xts[b], "wq": wq_s[hg], "wk": wk_s[hg], "wv": wv_s[hg],
            "wo": wo_s[hg], "ccat": ccat, "scat": scat, "mask": mask,
        })
    return in_maps


def kernel(x, Wq, Wk, Wv, Wo):
    global _NC, LAST_EXEC_NS
    from concourse.bass_utils import run_bass_kernel_spmd

    profile = bool(os.environ.get("KERNEL_PROFILE"))
    if profile:
        try:
            import hook_util
            hook_util.install()
            hook_util.patch_upload()
        except ImportError:
            profile = False

    in_maps = _prep_inputs(x, Wq, Wk, Wv, Wo)
    if _NC is None:
        _NC = _build_nc()

    kwargs = {}
    if profile:
        kwargs["tmpdir"] = os.environ.get("KERNEL_TRACE_DIR") or None
    res = run_bass_kernel_spmd(
        _NC, in_maps, core_ids=list(range(8)), trace=profile, **kwargs
    )
    LAST_EXEC_NS = res.exec_time_ns

    out = np.zeros((B, T, DM), dtype=np.float32)
    for core in range(8):
        out[core // 4] += res.results[core]["y"]
    return out


# revision 7
# speedup vs baseline: 1.0718x; 1.0014x over previous
"""Chunked sliding-window attention (B=2, T=8192, H=16, Dh=128, W=256) on 8
Trainium2 NeuronCores.

Sharding: 8 cores = 2 (batch) x 4 (head groups of 4 heads). Each core computes
q/k/v projections for its 512-wide slice of the 2048 projection dims, RoPE,
chunked attention for its 4 heads, and a partial output projection over its
512 rows of Wo^T. The host sums the 4 partial outputs per batch element.

Device layouts (host-prepared):
  xt   [128, 16, T]   x^T tiles: xt[p, kt, t] = x[b, t, kt*128+p]        (bf16)
  wq/wk[128, 16, 512] (Wq_perm)^T slice, rope-split row permutation      (bf16)
  wv   [128, 16, 512] Wv^T slice (unpermuted)                            (bf16)
  wo   [128, 4, 2048] Wo^T rows for this core's 512 dims                 (bf16)
  ccat [128, T]       [cos; cos] rope table (freq idx on partitions)     (bf16)
  scat [128, T]       [-sin; sin]                                        (bf16)
  mask [128, 384]     causal masks: cols 0:256 own-chunk kv tile 0,
                      cols 256:384 own-chunk kv tile 1 vs q-half b       (bf16)

Attention is computed in transposed-score layout [kv, q].  Per (head, chunk):
scores for the 4 kv tiles land pairwise in two [128,512] PSUM banks (the last
own-chunk kv tile only against the upper q-half - the lower half is fully
causal-masked), one exp per pair, one fused mask multiply, then the softmax
denominator comes from a single all-ones stationary matmul over the
DVE/GpSimd-summed exp tiles.  Output projection chains for the previous block
are interleaved into the attention units as PE filler so the engine never
waits on the exp->mask->sum chain.
"""

import os
from collections import deque

import numpy as np
import ml_dtypes

N_HEAD = 16
HEAD_DIM = 128
WINDOW = 256
THETA = 10000.0
B = 2
T = 8192
DM = 2048
KT = DM // 128      # 16 contraction tiles
HS = 4              # heads per core
DS = HS * HEAD_DIM  # 512 projection dims per core
BLK = 512           # tokens per pipeline block (2 chunks)
CH = WINDOW         # 256
SCALE = float(HEAD_DIM) ** -0.5

LAST_EXEC_NS = None
_NC = None

bf16 = ml_dtypes.bfloat16


def _build_nc(t_len=T):
    from contextlib import ExitStack

    import concourse.tile as tile
    from concourse import bacc, mybir

    fp32 = mybir.dt.float32
    b16 = mybir.dt.bfloat16

    nc = bacc.Bacc("TRN2", target_bir_lowering=False, debug=False)

    nb = t_len // BLK
    xt = nc.dram_tensor(
        "xt", [nb, 128, KT, BLK], b16, kind="ExternalInput"
    ).ap()
    wq = nc.dram_tensor("wq", [128, KT, DS], b16, kind="ExternalInput").ap()
    wk = nc.dram_tensor("wk", [128, KT, DS], b16, kind="ExternalInput").ap()
    wv = nc.dram_tensor("wv", [128, KT, DS], b16, kind="ExternalInput").ap()
    wo = nc.dram_tensor("wo", [128, HS, DM], b16, kind="ExternalInput").ap()
    ccat = nc.dram_tensor("ccat", [128, t_len], b16, kind="ExternalInput").ap()
    scat = nc.dram_tensor("scat", [128, t_len], b16, kind="ExternalInput").ap()
    mask = nc.dram_tensor("mask", [128, 384], b16, kind="ExternalInput").ap()
    y = nc.dram_tensor("y", [t_len, DM], fp32, kind="ExternalOutput").ap()

    Exp = mybir.ActivationFunctionType.Exp

    with tile.TileContext(nc) as tc, ExitStack() as ctx:
        const = ctx.enter_context(tc.tile_pool(name="const", bufs=1))
        xt_p = ctx.enter_context(tc.tile_pool(name="xtp", bufs=2))
        raw_p = ctx.enter_context(tc.tile_pool(name="rawp", bufs=3))
        swp_p = ctx.enter_context(tc.tile_pool(name="swpp", bufs=3))
        tmp_p = ctx.enter_context(tc.tile_pool(name="tmpp", bufs=3))
        qr_p = ctx.enter_context(tc.tile_pool(name="qrp", bufs=8))
        kr_p = ctx.enter_context(tc.tile_pool(name="krp", bufs=10))
        v_p = ctx.enter_context(tc.tile_pool(name="vp", bufs=10))
        e01_p = ctx.enter_context(tc.tile_pool(name="e01p", bufs=5))
        e23_p = ctx.enter_context(tc.tile_pool(name="e23p", bufs=5))
        es_p = ctx.enter_context(tc.tile_pool(name="esp", bufs=4))
        rc_p = ctx.enter_context(tc.tile_pool(name="rcp", bufs=4))
        ot_p = ctx.enter_context(tc.tile_pool(name="otp", bufs=20))
        y_p = ctx.enter_context(tc.tile_pool(name="yp", bufs=3))
        tab_p = ctx.enter_context(tc.tile_pool(name="tabp", bufs=3))
        # PSUM: 8 banks of [128,512]f32.  3 shared by proj+oproj chains,
        # 3 for the score pairs, 2 for the (AV out | denominator) pairs.
        ps_big = ctx.enter_context(tc.tile_pool(name="psbig", bufs=3, space="PSUM"))
        ps_sc = ctx.enter_context(tc.tile_pool(name="pssc", bufs=3, space="PSUM"))
        ps_od = ctx.enter_context(tc.tile_pool(name="psod", bufs=2, space="PSUM"))

        wq_sb = const.tile([128, KT, DS], b16)
        wk_sb = const.tile([128, KT, DS], b16)
        wv_sb = const.tile([128, KT, DS], b16)
        wo_sb = const.tile([128, HS, DM], b16)
        mask_sb = const.tile([128, 384], b16)
        ones_sb = const.tile([128, 128], b16)
        nc.vector.memset(ones_sb, 1.0)
        # First gpsimd tensor op pays a ~6us IRAM ucode load; pay it here,
        # during the const DMAs, instead of inside block 0's attention.
        warm_sb = const.tile([128, 8], b16)
        nc.gpsimd.memset(warm_sb, 0.0)
        nc.gpsimd.tensor_add(warm_sb, warm_sb, warm_sb)

        def fetch_tables(t0):
            cc = tab_p.tile([128, BLK], b16, tag="cc")
            nc.sync.dma_start(cc, ccat[:, t0 : t0 + BLK])
            sc = tab_p.tile([128, BLK], b16, tag="sc")
            nc.sync.dma_start(sc, scat[:, t0 : t0 + BLK])
            return cc, sc

        # Const loads interleaved so the first projection chain starts after
        # ~0.5 MB instead of the full constant set.
        xt_first = xt_p.tile([128, KT, BLK], b16, tag="xt")
        nc.sync.dma_start(mask_sb, mask)
        nc.sync.dma_start(wq_sb[:, 0:1, :], wq[:, 0:1, :])
        nc.sync.dma_start(xt_first[:, 0:1, :], xt[0][:, 0:1, :])
        nc.sync.dma_start(wq_sb[:, 1:2, :], wq[:, 1:2, :])
        nc.sync.dma_start(xt_first[:, 1:2, :], xt[0][:, 1:2, :])
        nc.sync.dma_start(wq_sb[:, 2:4, :], wq[:, 2:4, :])
        nc.sync.dma_start(xt_first[:, 2:4, :], xt[0][:, 2:4, :])
        tab_first = fetch_tables(0)
        nc.sync.dma_start(wq_sb[:, 4:8, :], wq[:, 4:8, :])
        nc.sync.dma_start(xt_first[:, 4:8, :], xt[0][:, 4:8, :])
        nc.sync.dma_start(wq_sb[:, 8:KT, :], wq[:, 8:KT, :])
        nc.sync.dma_start(xt_first[:, 8:KT, :], xt[0][:, 8:KT, :])
        nc.sync.dma_start(wk_sb[:, 0:4, :], wk[:, 0:4, :])
        nc.sync.dma_start(wk_sb[:, 4:8, :], wk[:, 4:8, :])
        nc.sync.dma_start(wk_sb[:, 8:KT, :], wk[:, 8:KT, :])
        nc.sync.dma_start(wv_sb[:, 0:8, :], wv[:, 0:8, :])
        nc.sync.dma_start(wv_sb[:, 8:KT, :], wv[:, 8:KT, :])
        tab_second = fetch_tables(BLK) if nb > 1 else None
        xt_second = None
        if nb > 1:
            xt_second = xt_p.tile([128, KT, BLK], b16, tag="xt")
            nc.sync.dma_start(xt_second, xt[1])
        for h in range(HS):
            nc.sync.dma_start(wo_sb[:, h, :], wo[:, h, :])

        def make_proj(xt_sb, c_sl, s_sl, qk_first=False):
            """Returns (thunks, cur_q, cur_k, cur_v); each thunk emits one
            PE chain (+rope / cast tail)."""
            cur_q = [None] * HS
            cur_k = [None] * HS
            cur_v = [None] * 4

            def qk_chain(h, w_sb, dst, pool, tag):
                def t():
                    ps = ps_big.tile([128, BLK], fp32, tag="psbig")
                    for k in range(KT):
                        nc.tensor.matmul(
                            ps,
                            lhsT=w_sb[:, k, h * 128 : (h + 1) * 128],
                            rhs=xt_sb[:, k, :],
                            start=(k == 0),
                            stop=(k == KT - 1),
                        )
                    raw = raw_p.tile([128, BLK], b16, tag="raw")
                    nc.scalar.copy(raw, ps)
                    # swap (re, im) halves via SBUF->SBUF DMA (DVE lanes
                    # cannot cross partitions)
                    swp = swp_p.tile([128, BLK], b16, tag="swp")
                    nc.sync.dma_start(swp[0:64, :], raw[64:128, :])
                    nc.sync.dma_start(swp[64:128, :], raw[0:64, :])
                    t1 = tmp_p.tile([128, BLK], b16, tag="t1")
                    nc.vector.tensor_mul(t1, raw, c_sl)
                    t2 = tmp_p.tile([128, BLK], b16, tag="t2")
                    nc.vector.tensor_mul(t2, swp, s_sl)
                    rot = pool.tile([128, BLK], b16, tag=tag)
                    nc.vector.tensor_add(rot, t1, t2)
                    dst[h] = rot
                return t

            def v_chain(tt):
                def t():
                    ps = ps_big.tile([128, BLK], fp32, tag="psbig")
                    for k in range(KT):
                        nc.tensor.matmul(
                            ps,
                            lhsT=xt_sb[:, k, tt * 128 : (tt + 1) * 128],
                            rhs=wv_sb[:, k, :],
                            start=(k == 0),
                            stop=(k == KT - 1),
                        )
                    vt = v_p.tile([128, DS], b16, tag="v")
                    nc.vector.tensor_copy(out=vt, in_=ps)
                    cur_v[tt] = vt
                return t

            thunks = []
            if qk_first:
                # block 0: wk arrives after wq+xt, so run all q chains first
                for h in range(HS):
                    thunks.append(qk_chain(h, wq_sb, cur_q, qr_p, "qr"))
                for h in range(HS):
                    thunks.append(qk_chain(h, wk_sb, cur_k, kr_p, "kr"))
            else:
                for h in range(HS):
                    thunks.append(qk_chain(h, wq_sb, cur_q, qr_p, "qr"))
                    thunks.append(qk_chain(h, wk_sb, cur_k, kr_p, "kr"))
            for tt in range(4):
                thunks.append(v_chain(tt))
            return thunks, cur_q, cur_k, cur_v

        def make_oproj_thunks(ot_map, base_t0, split_dma=False):
            """16 thunks: one (tt, ct) oproj chain each; DMA per tt tile."""
            ysbs = {}
            thunks = []
            for tt in range(4):
                ci, sub = tt // 2, tt % 2
                for ct in range(4):
                    def t(tt=tt, ct=ct, ci=ci, sub=sub):
                        if ct == 0:
                            ysbs[tt] = y_p.tile(
                                [128, DM], fp32, tag="y", name="ysb"
                            )
                        ysb = ysbs[tt]
                        yps = ps_big.tile([128, 512], fp32, tag="psbig")
                        for h in range(HS):
                            nc.tensor.matmul(
                                yps,
                                lhsT=ot_map[(h, ci)][:, sub * 128 : (sub + 1) * 128],
                                rhs=wo_sb[:, h, ct * 512 : (ct + 1) * 512],
                                start=(h == 0),
                                stop=(h == HS - 1),
                            )
                        if ct % 2 == 0:
                            nc.scalar.copy(ysb[:, ct * 512 : (ct + 1) * 512], yps)
                        else:
                            nc.vector.tensor_copy(
                                out=ysb[:, ct * 512 : (ct + 1) * 512], in_=yps
                            )
                        rows = slice(base_t0 + tt * 128, base_t0 + (tt + 1) * 128)
                        if split_dma and ct == 1:
                            nc.sync.dma_start(y[rows, 0:1024], ysb[:, 0:1024])
                        elif split_dma and ct == 3:
                            nc.sync.dma_start(y[rows, 1024:DM], ysb[:, 1024:DM])
                        elif not split_dma and ct == 3:
                            nc.sync.dma_start(y[rows, :], ysb)
                    thunks.append(t)
            return thunks

        class Unit:
            """One (chunk-half, head) attention unit."""

            def __init__(self, blk, ci, h, cur_q, cur_k, cur_v, prev_k, prev_v):
                self.ci, self.h = ci, h
                self.c = 2 * blk + ci
                self.qoff = ci * CH
                self.full = self.c != 0
                self.cur_q, self.cur_k, self.cur_v = cur_q, cur_k, cur_v
                self.prev_k, self.prev_v = prev_k, prev_v

            def ksrc(self, j):
                h, ci, qoff = self.h, self.ci, self.qoff
                if j < 2:
                    if ci == 1:
                        return self.cur_k[h][:, j * 128 : (j + 1) * 128]
                    return self.prev_k[h][:, CH + j * 128 : CH + (j + 1) * 128]
                return self.cur_k[h][:, qoff + (j - 2) * 128 : qoff + (j - 1) * 128]

            def vsrc(self, j):
                if j < 2:
                    vt = self.cur_v[j] if self.ci == 1 else self.prev_v[j]
                else:
                    vt = self.cur_v[2 * self.ci + (j - 2)]
                return vt[:, self.h * 128 : (self.h + 1) * 128]

            def emit_scores(self):
                q_sl = self.cur_q[self.h][:, self.qoff : self.qoff + CH]
                if self.full:
                    st01 = ps_sc.tile([128, 512], fp32, tag="sc")
                    for j in (0, 1):
                        nc.tensor.matmul(
                            st01[:, j * CH : (j + 1) * CH],
                            lhsT=self.ksrc(j), rhs=q_sl,
                            start=True, stop=True, skip_group_check=True,
                        )
                    e01 = e01_p.tile([128, 512], b16, tag="e01")
                    nc.scalar.activation(e01, st01, Exp, scale=SCALE)
                    self.e01 = e01
                st23 = ps_sc.tile([128, 512], fp32, tag="sc")
                nc.tensor.matmul(
                    st23[:, 0:CH], lhsT=self.ksrc(2), rhs=q_sl,
                    start=True, stop=True, skip_group_check=True,
                )
                # last own-chunk kv tile: lower q-half fully causal-masked
                nc.tensor.matmul(
                    st23[:, CH : CH + 128], lhsT=self.ksrc(3),
                    rhs=q_sl[:, 128:CH],
                    start=True, stop=True, skip_group_check=True,
                )
                e23 = e23_p.tile([128, 384], b16, tag="e23")
                nc.scalar.activation(e23, st23[:, 0:384], Exp, scale=SCALE)
                self.e23 = e23
                if self.full:
                    es = es_p.tile([128, CH], b16, tag="es")
                    nc.vector.tensor_add(es, e01[:, 0:CH], e01[:, CH:512])
                    self.es = es
                else:
                    self.es = None
                nc.vector.tensor_mul(e23, e23, mask_sb)
                if self.full:
                    nc.gpsimd.tensor_add(self.es, self.es, e23[:, 0:CH])
                    nc.gpsimd.tensor_add(
                        self.es[:, 128:CH], self.es[:, 128:CH], e23[:, CH:384]
                    )

            def emit_avdn(self, ot_map):
                od = ps_od.tile([128, 512], fp32, tag="od")
                ou = od[:, 0:CH]
                dn = od[:, CH:512]
                if self.full:
                    nc.tensor.matmul(
                        ou, lhsT=self.vsrc(0), rhs=self.e01[:, 0:CH],
                        start=True, stop=False, skip_group_check=True,
                    )
                    nc.tensor.matmul(
                        ou, lhsT=self.vsrc(1), rhs=self.e01[:, CH:512],
                        start=False, stop=False, skip_group_check=True,
                    )
                    nc.tensor.matmul(
                        ou, lhsT=self.vsrc(2), rhs=self.e23[:, 0:CH],
                        start=False, stop=False, skip_group_check=True,
                    )
                else:
                    nc.tensor.matmul(
                        ou, lhsT=self.vsrc(2), rhs=self.e23[:, 0:CH],
                        start=True, stop=False, skip_group_check=True,
                    )
                nc.tensor.matmul(
                    ou[:, 128:CH], lhsT=self.vsrc(3), rhs=self.e23[:, CH:384],
                    start=False, stop=True, skip_group_check=True,
                )
                if self.es is not None:
                    nc.tensor.matmul(
                        dn, lhsT=ones_sb, rhs=self.es,
                        start=True, stop=True, skip_group_check=True,
                    )
                else:
                    nc.tensor.matmul(
                        dn, lhsT=ones_sb, rhs=self.e23[:, 0:CH],
                        start=True, stop=False, skip_group_check=True,
                    )
                    nc.tensor.matmul(
                        dn[:, 128:CH], lhsT=ones_sb, rhs=self.e23[:, CH:384],
                        start=False, stop=True, skip_group_check=True,
                    )
                rc = rc_p.tile([128, CH], fp32, tag="rc")
                nc.vector.reciprocal_approx_fast(out=rc, in_=dn)
                ot = ot_p.tile([128, CH], b16, tag="ot")
                nc.vector.tensor_mul(ot, ou, rc)
                ot_map[(self.h, self.ci)] = ot

        prev_k = [None] * HS
        prev_v = [None, None]
        pend = None  # (ot_map, t0) of previous block
        pending_proj = None  # blk1's projections, emitted during blk0's att
        carry = None  # last unit of previous block: avdn deferred into this one

        for blk in range(nb):
            t0 = blk * BLK
            if blk == 0:
                xt_sb, (c_sl, s_sl) = xt_first, tab_first
            elif blk == 1:
                xt_sb, (c_sl, s_sl) = xt_second, tab_second
            else:
                xt_sb = xt_p.tile([128, KT, BLK], b16, tag="xt")
                nc.sync.dma_start(xt_sb, xt[blk])
                c_sl, s_sl = fetch_tables(t0)

            if pending_proj is not None:
                cur_q, cur_k, cur_v = pending_proj
                pending_proj = None
                if carry is not None:
                    carry[0].emit_avdn(carry[1])
                    carry = None
            else:
                thunks, cur_q, cur_k, cur_v = make_proj(
                    xt_sb, c_sl, s_sl, qk_first=(blk == 0)
                )
                thunks[0]()
                if carry is not None:
                    carry[0].emit_avdn(carry[1])
                    carry = None
                for t in thunks[1:]:
                    t()

            fillers = deque()
            if pend is not None:
                fillers.extend(make_oproj_thunks(*pend))
            if blk == 0 and nb > 1:
                thunks1, cq1, ck1, cv1 = make_proj(xt_second, *tab_second)
                fillers.extend(thunks1)
                pending_proj = (cq1, ck1, cv1)

            last = blk == nb - 1
            quota = 3 if last else 2
            ot_map = {}
            own = make_oproj_thunks(ot_map, t0, split_dma=True) if last else None

            units = [
                Unit(blk, ci, h, cur_q, cur_k, cur_v, prev_k, prev_v)
                for ci in range(2)
                for h in range(HS)
            ]
            prev_u = None
            for ui, u in enumerate(units):
                u.emit_scores()
                for _ in range(quota):
                    if fillers:
                        fillers.popleft()()
                if prev_u is not None:
                    prev_u.emit_avdn(ot_map)
                if last and ui == 4:
                    # ci0's ot tiles complete after the ui==4 emit_avdn;
                    # its oproj chains become eligible filler
                    fillers.extend(own[:8])
                prev_u = u
            if last:
                units[-1].emit_avdn(ot_map)
                while fillers:
                    fillers.popleft()()
                for t in own[8:]:
                    t()
            else:
                while fillers:
                    fillers.popleft()()
                carry = (units[-1], ot_map)
                pend = (ot_map, t0)

            prev_k = cur_k
            prev_v = cur_v[2:4]

    nc.compile()
    return nc


def _rope_perm():
    perm = np.empty(DM, np.int64)
    for h in range(N_HEAD):
        base = h * HEAD_DIM
        perm[base : base + 64] = base + 2 * np.arange(64)
        perm[base + 64 : base + 128] = base + 2 * np.arange(64) + 1
    return perm


def _prep_inputs(x, Wq, Wk, Wv, Wo, t_len=T):
    """Build per-core in_maps. Cores 0-3: batch 0, head groups 0-3; 4-7: batch 1."""
    x = np.asarray(x, dtype=np.float32)
    Wq = np.asarray(Wq, dtype=np.float32)
    Wk = np.asarray(Wk, dtype=np.float32)
    Wv = np.asarray(Wv, dtype=np.float32)
    Wo = np.asarray(Wo, dtype=np.float32)
    nb_b = x.shape[0]

    perm = _rope_perm()
    wqT = np.ascontiguousarray(Wq[perm].T).astype(bf16)  # [K, dout_perm]
    wkT = np.ascontiguousarray(Wk[perm].T).astype(bf16)
    wvT = np.ascontiguousarray(Wv.T).astype(bf16)
    woT = np.ascontiguousarray(Wo.T).astype(bf16)        # [d, c]

    # xt[blk, p, kt, t_in_blk] = x[b, blk*BLK + t, kt*128+p] — block-major so
    # each block's slab is one fully-contiguous DMA read per partition
    nblk = t_len // BLK
    xts = []
    for b in range(nb_b):
        xT = x[b].T.reshape(KT, 128, nblk, BLK)
        xts.append(np.ascontiguousarray(xT.transpose(2, 1, 0, 3)).astype(bf16))

    wq_s, wk_s, wv_s, wo_s = [], [], [], []
    for hg in range(4):
        sl = slice(hg * DS, (hg + 1) * DS)
        wq_s.append(np.ascontiguousarray(
            wqT[:, sl].reshape(KT, 128, DS).transpose(1, 0, 2)).astype(bf16))
        wk_s.append(np.ascontiguousarray(
            wkT[:, sl].reshape(KT, 128, DS).transpose(1, 0, 2)).astype(bf16))
        wv_s.append(np.ascontiguousarray(
            wvT[:, sl].reshape(KT, 128, DS).transpose(1, 0, 2)).astype(bf16))
        wo_s.append(np.ascontiguousarray(
            woT[sl].reshape(HS, 128, DM).transpose(1, 0, 2)).astype(bf16))

    inv = 1.0 / THETA ** (np.arange(0, HEAD_DIM, 2, dtype=np.float32) / HEAD_DIM)
    fr = np.outer(inv, np.arange(t_len, dtype=np.float32))  # [64, T]
    cosT = np.cos(fr).astype(np.float32)
    sinT = np.sin(fr).astype(np.float32)
    ccat = np.concatenate([cosT, cosT], axis=0).astype(bf16)   # [128, T]
    scat = np.concatenate([-sinT, sinT], axis=0).astype(bf16)  # [128, T]

    r = np.arange(128)[:, None]
    qc = np.arange(CH)[None, :]
    qh = np.arange(128)[None, :]
    mask = np.concatenate(
        [(r <= qc), (r <= qh)], axis=1
    ).astype(bf16)  # [128, 384]: own kv tile 0 vs all q; own kv tile 1 vs q-half b

    in_maps = []
    for core in range(8):
        b, hg = core // 4, core % 4
        in_maps.append({
            "xt": xts[b], "wq": wq_s[hg], "wk": wk_s[hg], "wv": wv_s[hg],
            "wo": wo_s[hg], "ccat": ccat, "scat": scat, "mask": mask,
        })
    return in_maps


def kernel(x, Wq, Wk, Wv, Wo):
    global _NC, LAST_EXEC_NS
    from concourse.bass_utils import run_bass_kernel_spmd

    profile = bool(os.environ.get("KERNEL_PROFILE"))
    if profile:
        try:
            import hook_util
            hook_util.install()
            hook_util.patch_upload()
        except ImportError:
            profile = False

    in_maps = _prep_inputs(x, Wq, Wk, Wv, Wo)
    if _NC is None:
        _NC = _build_nc()

    kwargs = {}
    if profile:
        kwargs["tmpdir"] = os.environ.get("KERNEL_TRACE_DIR") or None
    res = run_bass_kernel_spmd(
        _NC, in_maps, core_ids=list(range(8)), trace=profile, **kwargs
    )
    LAST_EXEC_NS = res.exec_time_ns

    out = np.zeros((B, T, DM), dtype=np.float32)
    for core in range(8):
        out[core // 4] += res.results[core]["y"]
    return out


# revision 10
# speedup vs baseline: 1.0862x; 1.0135x over previous
"""Chunked sliding-window attention (B=2, T=8192, H=16, Dh=128, W=256) on 8
Trainium2 NeuronCores.

Sharding: 8 cores = 2 (batch) x 4 (head groups of 4 heads). Each core computes
q/k/v projections for its 512-wide slice of the 2048 projection dims, RoPE,
chunked attention for its 4 heads, and a partial output projection over its
512 rows of Wo^T. The host sums the 4 partial outputs per batch element.

Device layouts (host-prepared):
  xt   [128, 16, T]   x^T tiles: xt[p, kt, t] = x[b, t, kt*128+p]        (bf16)
  wq/wk[128, 16, 512] (Wq_perm)^T slice, rope-split row permutation      (bf16)
  wv   [128, 16, 512] Wv^T slice (unpermuted)                            (bf16)
  wo   [128, 4, 2048] Wo^T rows for this core's 512 dims                 (bf16)
  ccat [128, T]       [cos; cos] rope table (freq idx on partitions)     (bf16)
  scat [128, T]       [-sin; sin]                                        (bf16)
  mask [128, 384]     causal masks: cols 0:256 own-chunk kv tile 0,
                      cols 256:384 own-chunk kv tile 1 vs q-half b       (bf16)

Attention is computed in transposed-score layout [kv, q].  Per (head, chunk):
scores for the 4 kv tiles land pairwise in two [128,512] PSUM banks (the last
own-chunk kv tile only against the upper q-half - the lower half is fully
causal-masked), one exp per pair, one fused mask multiply, then the softmax
denominator comes from a single all-ones stationary matmul over the
DVE/GpSimd-summed exp tiles.  Output projection chains for the previous block
are interleaved into the attention units as PE filler so the engine never
waits on the exp->mask->sum chain.
"""

import os
from collections import deque

import numpy as np
import ml_dtypes

N_HEAD = 16
HEAD_DIM = 128
WINDOW = 256
THETA = 10000.0
B = 2
T = 8192
DM = 2048
KT = DM // 128      # 16 contraction tiles
HS = 4              # heads per core
DS = HS * HEAD_DIM  # 512 projection dims per core
BLK = 512           # tokens per pipeline block (2 chunks)
CH = WINDOW         # 256
SCALE = float(HEAD_DIM) ** -0.5

LAST_EXEC_NS = None
_NC = None

bf16 = ml_dtypes.bfloat16


def _build_nc(t_len=T):
    from contextlib import ExitStack

    import concourse.tile as tile
    from concourse import bacc, mybir

    fp32 = mybir.dt.float32
    b16 = mybir.dt.bfloat16

    nc = bacc.Bacc("TRN2", target_bir_lowering=False, debug=False)

    nb = t_len // BLK
    xt = nc.dram_tensor(
        "xt", [nb, 128, KT, BLK], b16, kind="ExternalInput"
    ).ap()
    wq = nc.dram_tensor("wq", [128, KT, DS], b16, kind="ExternalInput").ap()
    wk = nc.dram_tensor("wk", [128, KT, DS], b16, kind="ExternalInput").ap()
    wv = nc.dram_tensor("wv", [128, KT, DS], b16, kind="ExternalInput").ap()
    wo = nc.dram_tensor("wo", [128, HS, DM], b16, kind="ExternalInput").ap()
    ccat = nc.dram_tensor("ccat", [128, t_len], b16, kind="ExternalInput").ap()
    scat = nc.dram_tensor("scat", [128, t_len], b16, kind="ExternalInput").ap()
    mask = nc.dram_tensor("mask", [128, 384], b16, kind="ExternalInput").ap()
    y = nc.dram_tensor("y", [t_len, DM], fp32, kind="ExternalOutput").ap()

    Exp = mybir.ActivationFunctionType.Exp

    with tile.TileContext(nc) as tc, ExitStack() as ctx:
        const = ctx.enter_context(tc.tile_pool(name="const", bufs=1))
        xt_p = ctx.enter_context(tc.tile_pool(name="xtp", bufs=2))
        # raw/swp rotation must ride out the startup window where the small
        # rope-swap DMAs queue behind ~12 MB of const loads: deep pools keep
        # the scalar/vector queues from blocking on buffer reuse.
        raw_p = ctx.enter_context(tc.tile_pool(name="rawp", bufs=10))
        swp_p = ctx.enter_context(tc.tile_pool(name="swpp", bufs=8))
        tmp_p = ctx.enter_context(tc.tile_pool(name="tmpp", bufs=6))
        qr_p = ctx.enter_context(tc.tile_pool(name="qrp", bufs=8))
        kr_p = ctx.enter_context(tc.tile_pool(name="krp", bufs=10))
        v_p = ctx.enter_context(tc.tile_pool(name="vp", bufs=10))
        e01_p = ctx.enter_context(tc.tile_pool(name="e01p", bufs=5))
        e23_p = ctx.enter_context(tc.tile_pool(name="e23p", bufs=5))
        es_p = ctx.enter_context(tc.tile_pool(name="esp", bufs=4))
        rc_p = ctx.enter_context(tc.tile_pool(name="rcp", bufs=4))
        ot_p = ctx.enter_context(tc.tile_pool(name="otp", bufs=20))
        y_p = ctx.enter_context(tc.tile_pool(name="yp", bufs=2))
        tab_p = ctx.enter_context(tc.tile_pool(name="tabp", bufs=3))
        # PSUM: 8 banks of [128,512]f32.  3 shared by proj+oproj chains,
        # 3 for the score pairs, 2 for the (AV out | denominator) pairs.
        ps_big = ctx.enter_context(tc.tile_pool(name="psbig", bufs=3, space="PSUM"))
        ps_sc = ctx.enter_context(tc.tile_pool(name="pssc", bufs=3, space="PSUM"))
        ps_od = ctx.enter_context(tc.tile_pool(name="psod", bufs=2, space="PSUM"))

        wq_sb = const.tile([128, KT, DS], b16)
        wk_sb = const.tile([128, KT, DS], b16)
        wv_sb = const.tile([128, KT, DS], b16)
        wo_sb = const.tile([128, HS, DM], b16)
        mask_sb = const.tile([128, 384], b16)
        ones_sb = const.tile([128, 128], b16)
        nc.vector.memset(ones_sb, 1.0)
        # First gpsimd tensor op pays a ~6us IRAM ucode load; pay it here,
        # during the const DMAs, instead of inside block 0's attention.
        warm_sb = const.tile([128, 8], b16)
        nc.gpsimd.memset(warm_sb, 0.0)
        nc.gpsimd.tensor_add(warm_sb, warm_sb, warm_sb)

        def fetch_tables(t0):
            cc = tab_p.tile([128, BLK], b16, tag="cc")
            nc.sync.dma_start(cc, ccat[:, t0 : t0 + BLK])
            sc = tab_p.tile([128, BLK], b16, tag="sc")
            nc.sync.dma_start(sc, scat[:, t0 : t0 + BLK])
            return cc, sc

        # Const loads interleaved so the first projection chain starts after
        # ~0.5 MB instead of the full constant set.
        xt_first = xt_p.tile([128, KT, BLK], b16, tag="xt")
        nc.sync.dma_start(mask_sb, mask)
        nc.sync.dma_start(wq_sb[:, 0:4, :], wq[:, 0:4, :])
        nc.sync.dma_start(xt_first[:, 0:4, :], xt[0][:, 0:4, :])
        tab_first = fetch_tables(0)
        nc.sync.dma_start(wq_sb[:, 4:8, :], wq[:, 4:8, :])
        nc.sync.dma_start(xt_first[:, 4:8, :], xt[0][:, 4:8, :])
        nc.sync.dma_start(wq_sb[:, 8:KT, :], wq[:, 8:KT, :])
        nc.sync.dma_start(xt_first[:, 8:KT, :], xt[0][:, 8:KT, :])
        nc.sync.dma_start(wk_sb[:, 0:4, :], wk[:, 0:4, :])
        nc.sync.dma_start(wk_sb[:, 4:8, :], wk[:, 4:8, :])
        nc.sync.dma_start(wk_sb[:, 8:KT, :], wk[:, 8:KT, :])
        nc.sync.dma_start(wv_sb[:, 0:8, :], wv[:, 0:8, :])
        nc.sync.dma_start(wv_sb[:, 8:KT, :], wv[:, 8:KT, :])
        tab_second = fetch_tables(BLK) if nb > 1 else None
        xt_second = None
        if nb > 1:
            xt_second = xt_p.tile([128, KT, BLK], b16, tag="xt")
            nc.sync.dma_start(xt_second, xt[1])
        for h in range(HS):
            nc.sync.dma_start(wo_sb[:, h, :], wo[:, h, :])

        def make_proj(xt_sb, c_sl, s_sl, qk_first=False):
            """Returns (thunks, cur_q, cur_k, cur_v); each thunk emits one
            PE chain (+rope / cast tail)."""
            cur_q = [None] * HS
            cur_k = [None] * HS
            cur_v = [None] * 4

            def qk_chain(h, w_sb, dst, pool, tag):
                def t():
                    ps = ps_big.tile([128, BLK], fp32, tag="psbig")
                    for k in range(KT):
                        nc.tensor.matmul(
                            ps,
                            lhsT=w_sb[:, k, h * 128 : (h + 1) * 128],
                            rhs=xt_sb[:, k, :],
                            start=(k == 0),
                            stop=(k == KT - 1),
                        )
                    raw = raw_p.tile([128, BLK], b16, tag="raw")
                    nc.scalar.copy(raw, ps)
                    # swap (re, im) halves via SBUF->SBUF DMA (DVE lanes
                    # cannot cross partitions)
                    swp = swp_p.tile([128, BLK], b16, tag="swp")
                    nc.sync.dma_start(swp[0:64, :], raw[64:128, :])
                    nc.sync.dma_start(swp[64:128, :], raw[0:64, :])
                    t1 = tmp_p.tile([128, BLK], b16, tag="t1")
                    nc.vector.tensor_mul(t1, raw, c_sl)
                    t2 = tmp_p.tile([128, BLK], b16, tag="t2")
                    nc.vector.tensor_mul(t2, swp, s_sl)
                    rot = pool.tile([128, BLK], b16, tag=tag)
                    nc.vector.tensor_add(rot, t1, t2)
                    dst[h] = rot
                return t

            def v_chain(tt):
                def t():
                    ps = ps_big.tile([128, BLK], fp32, tag="psbig")
                    for k in range(KT):
                        nc.tensor.matmul(
                            ps,
                            lhsT=xt_sb[:, k, tt * 128 : (tt + 1) * 128],
                            rhs=wv_sb[:, k, :],
                            start=(k == 0),
                            stop=(k == KT - 1),
                        )
                    vt = v_p.tile([128, DS], b16, tag="v")
                    nc.vector.tensor_copy(out=vt, in_=ps)
                    cur_v[tt] = vt
                return t

            thunks = []
            if qk_first:
                # block 0: wk arrives after wq+xt, so run all q chains first
                for h in range(HS):
                    thunks.append(qk_chain(h, wq_sb, cur_q, qr_p, "qr"))
                for h in range(HS):
                    thunks.append(qk_chain(h, wk_sb, cur_k, kr_p, "kr"))
            else:
                for h in range(HS):
                    thunks.append(qk_chain(h, wq_sb, cur_q, qr_p, "qr"))
                    thunks.append(qk_chain(h, wk_sb, cur_k, kr_p, "kr"))
            for tt in range(4):
                thunks.append(v_chain(tt))
            return thunks, cur_q, cur_k, cur_v

        def make_oproj_thunks(ot_map, base_t0, split_dma=False):
            """16 thunks: one (tt, ct) oproj chain each; DMA per tt tile."""
            ysbs = {}
            thunks = []
            for tt in range(4):
                ci, sub = tt // 2, tt % 2
                for ct in range(4):
                    def t(tt=tt, ct=ct, ci=ci, sub=sub):
                        if ct == 0:
                            ysbs[tt] = y_p.tile(
                                [128, DM], fp32, tag="y", name="ysb"
                            )
                        ysb = ysbs[tt]
                        yps = ps_big.tile([128, 512], fp32, tag="psbig")
                        for h in range(HS):
                            nc.tensor.matmul(
                                yps,
                                lhsT=ot_map[(h, ci)][:, sub * 128 : (sub + 1) * 128],
                                rhs=wo_sb[:, h, ct * 512 : (ct + 1) * 512],
                                start=(h == 0),
                                stop=(h == HS - 1),
                            )
                        if ct % 2 == 0:
                            nc.scalar.copy(ysb[:, ct * 512 : (ct + 1) * 512], yps)
                        else:
                            nc.vector.tensor_copy(
                                out=ysb[:, ct * 512 : (ct + 1) * 512], in_=yps
                            )
                        rows = slice(base_t0 + tt * 128, base_t0 + (tt + 1) * 128)
                        if split_dma and ct == 1:
                            nc.sync.dma_start(y[rows, 0:1024], ysb[:, 0:1024])
                        elif split_dma and ct == 3:
                            nc.sync.dma_start(y[rows, 1024:DM], ysb[:, 1024:DM])
                        elif not split_dma and ct == 3:
                            nc.sync.dma_start(y[rows, :], ysb)
                    thunks.append(t)
            return thunks

        class Unit:
            """One (chunk-half, head) attention unit."""

            def __init__(self, blk, ci, h, cur_q, cur_k, cur_v, prev_k, prev_v):
                self.ci, self.h = ci, h
                self.c = 2 * blk + ci
                self.qoff = ci * CH
                self.full = self.c != 0
                self.cur_q, self.cur_k, self.cur_v = cur_q, cur_k, cur_v
                self.prev_k, self.prev_v = prev_k, prev_v

            def ksrc(self, j):
                h, ci, qoff = self.h, self.ci, self.qoff
                if j < 2:
                    if ci == 1:
                        return self.cur_k[h][:, j * 128 : (j + 1) * 128]
                    return self.prev_k[h][:, CH + j * 128 : CH + (j + 1) * 128]
                return self.cur_k[h][:, qoff + (j - 2) * 128 : qoff + (j - 1) * 128]

            def vsrc(self, j):
                if j < 2:
                    vt = self.cur_v[j] if self.ci == 1 else self.prev_v[j]
                else:
                    vt = self.cur_v[2 * self.ci + (j - 2)]
                return vt[:, self.h * 128 : (self.h + 1) * 128]

            def emit_scores(self):
                q_sl = self.cur_q[self.h][:, self.qoff : self.qoff + CH]
                if self.full:
                    st01 = ps_sc.tile([128, 512], fp32, tag="sc")
                    for j in (0, 1):
                        nc.tensor.matmul(
                            st01[:, j * CH : (j + 1) * CH],
                            lhsT=self.ksrc(j), rhs=q_sl,
                            start=True, stop=True, skip_group_check=True,
                        )
                    e01 = e01_p.tile([128, 512], b16, tag="e01")
                    nc.scalar.activation(e01, st01, Exp, scale=SCALE)
                    self.e01 = e01
                st23 = ps_sc.tile([128, 512], fp32, tag="sc")
                nc.tensor.matmul(
                    st23[:, 0:CH], lhsT=self.ksrc(2), rhs=q_sl,
                    start=True, stop=True, skip_group_check=True,
                )
                # last own-chunk kv tile: lower q-half fully causal-masked
                nc.tensor.matmul(
                    st23[:, CH : CH + 128], lhsT=self.ksrc(3),
                    rhs=q_sl[:, 128:CH],
                    start=True, stop=True, skip_group_check=True,
                )
                e23 = e23_p.tile([128, 384], b16, tag="e23")
                nc.scalar.activation(e23, st23[:, 0:384], Exp, scale=SCALE)
                self.e23 = e23
                if self.full:
                    es = es_p.tile([128, CH], b16, tag="es")
                    nc.vector.tensor_add(es, e01[:, 0:CH], e01[:, CH:512])
                    self.es = es
                else:
                    self.es = None
                nc.vector.tensor_mul(e23, e23, mask_sb)
                if self.full:
                    nc.gpsimd.tensor_add(self.es, self.es, e23[:, 0:CH])
                    nc.gpsimd.tensor_add(
                        self.es[:, 128:CH], self.es[:, 128:CH], e23[:, CH:384]
                    )

            def emit_avdn(self, ot_map):
                od = ps_od.tile([128, 512], fp32, tag="od")
                ou = od[:, 0:CH]
                dn = od[:, CH:512]
                if self.full:
                    nc.tensor.matmul(
                        ou, lhsT=self.vsrc(0), rhs=self.e01[:, 0:CH],
                        start=True, stop=False, skip_group_check=True,
                    )
                    nc.tensor.matmul(
                        ou, lhsT=self.vsrc(1), rhs=self.e01[:, CH:512],
                        start=False, stop=False, skip_group_check=True,
                    )
                    nc.tensor.matmul(
                        ou, lhsT=self.vsrc(2), rhs=self.e23[:, 0:CH],
                        start=False, stop=False, skip_group_check=True,
                    )
                else:
                    nc.tensor.matmul(
                        ou, lhsT=self.vsrc(2), rhs=self.e23[:, 0:CH],
                        start=True, stop=False, skip_group_check=True,
                    )
                nc.tensor.matmul(
                    ou[:, 128:CH], lhsT=self.vsrc(3), rhs=self.e23[:, CH:384],
                    start=False, stop=True, skip_group_check=True,
                )
                if self.es is not None:
                    nc.tensor.matmul(
                        dn, lhsT=ones_sb, rhs=self.es,
                        start=True, stop=True, skip_group_check=True,
                    )
                else:
                    nc.tensor.matmul(
                        dn, lhsT=ones_sb, rhs=self.e23[:, 0:CH],
                        start=True, stop=False, skip_group_check=True,
                    )
                    nc.tensor.matmul(
                        dn[:, 128:CH], lhsT=ones_sb, rhs=self.e23[:, CH:384],
                        start=False, stop=True, skip_group_check=True,
                    )
                rc = rc_p.tile([128, CH], fp32, tag="rc")
                nc.vector.reciprocal_approx_fast(out=rc, in_=dn)
                ot = ot_p.tile([128, CH], b16, tag="ot")
                nc.vector.tensor_mul(ot, ou, rc)
                ot_map[(self.h, self.ci)] = ot

        prev_k = [None] * HS
        prev_v = [None, None]
        pend = None  # (ot_map, t0) of previous block
        pending_proj = None  # blk1's projections, emitted during blk0's att
        carry = None  # last unit of previous block: avdn deferred into this one

        for blk in range(nb):
            t0 = blk * BLK
            if blk == 0:
                xt_sb, (c_sl, s_sl) = xt_first, tab_first
            elif blk == 1:
                xt_sb, (c_sl, s_sl) = xt_second, tab_second
            else:
                xt_sb = xt_p.tile([128, KT, BLK], b16, tag="xt")
                nc.sync.dma_start(xt_sb, xt[blk])
                c_sl, s_sl = fetch_tables(t0)

            if pending_proj is not None:
                cur_q, cur_k, cur_v = pending_proj
                pending_proj = None
                if carry is not None:
                    carry[0].emit_avdn(carry[1])
                    carry = None
            else:
                thunks, cur_q, cur_k, cur_v = make_proj(
                    xt_sb, c_sl, s_sl, qk_first=(blk == 0)
                )
                thunks[0]()
                if carry is not None:
                    carry[0].emit_avdn(carry[1])
                    carry = None
                for t in thunks[1:]:
                    t()

            fillers = deque()
            if pend is not None:
                fillers.extend(make_oproj_thunks(*pend))
            if blk == 0 and nb > 1:
                thunks1, cq1, ck1, cv1 = make_proj(xt_second, *tab_second)
                fillers.extend(thunks1)
                pending_proj = (cq1, ck1, cv1)

            last = blk == nb - 1
            quota = 3 if last else 2
            ot_map = {}
            own = make_oproj_thunks(ot_map, t0, split_dma=True) if last else None

            units = [
                Unit(blk, ci, h, cur_q, cur_k, cur_v, prev_k, prev_v)
                for ci in range(2)
                for h in range(HS)
            ]
            prev_u = None
            for ui, u in enumerate(units):
                u.emit_scores()
                for _ in range(quota):
                    if fillers:
                        fillers.popleft()()
                if prev_u is not None:
                    prev_u.emit_avdn(ot_map)
                if last and ui == 4:
                    # ci0's ot tiles complete after the ui==4 emit_avdn;
                    # its oproj chains become eligible filler
                    fillers.extend(own[:8])
                prev_u = u
            if last:
                units[-1].emit_avdn(ot_map)
                while fillers:
                    fillers.popleft()()
                for t in own[8:]:
                    t()
            else:
                while fillers:
                    fillers.popleft()()
                carry = (units[-1], ot_map)
                pend = (ot_map, t0)

            prev_k = cur_k
            prev_v = cur_v[2:4]

    nc.compile()
    return nc


def _rope_perm():
    perm = np.empty(DM, np.int64)
    for h in range(N_HEAD):
        base = h * HEAD_DIM
        perm[base : base + 64] = base + 2 * np.arange(64)
        perm[base + 64 : base + 128] = base + 2 * np.arange(64) + 1
    return perm


def _prep_inputs(x, Wq, Wk, Wv, Wo, t_len=T):
    """Build per-core in_maps. Cores 0-3: batch 0, head groups 0-3; 4-7: batch 1."""
    x = np.asarray(x, dtype=np.float32)
    Wq = np.asarray(Wq, dtype=np.float32)
    Wk = np.asarray(Wk, dtype=np.float32)
    Wv = np.asarray(Wv, dtype=np.float32)
    Wo = np.asarray(Wo, dtype=np.float32)
    nb_b = x.shape[0]

    perm = _rope_perm()
    wqT = np.ascontiguousarray(Wq[perm].T).astype(bf16)  # [K, dout_perm]
    wkT = np.ascontiguousarray(Wk[perm].T).astype(bf16)
    wvT = np.ascontiguousarray(Wv.T).astype(bf16)
    woT = np.ascontiguousarray(Wo.T).astype(bf16)        # [d, c]

    # xt[blk, p, kt, t_in_blk] = x[b, blk*BLK + t, kt*128+p] — block-major so
    # each block's slab is one fully-contiguous DMA read per partition
    nblk = t_len // BLK
    xts = []
    for b in range(nb_b):
        xT = x[b].T.reshape(KT, 128, nblk, BLK)
        xts.append(np.ascontiguousarray(xT.transpose(2, 1, 0, 3)).astype(bf16))

    wq_s, wk_s, wv_s, wo_s = [], [], [], []
    for hg in range(4):
        sl = slice(hg * DS, (hg + 1) * DS)
        wq_s.append(np.ascontiguousarray(
            wqT[:, sl].reshape(KT, 128, DS).transpose(1, 0, 2)).astype(bf16))
        wk_s.append(np.ascontiguousarray(
            wkT[:, sl].reshape(KT, 128, DS).transpose(1, 0, 2)).astype(bf16))
        wv_s.append(np.ascontiguousarray(
            wvT[:, sl].reshape(KT, 128, DS).transpose(1, 0, 2)).astype(bf16))
        wo_s.append(np.ascontiguousarray(
            woT[sl].reshape(HS, 128, DM).transpose(1, 0, 2)).astype(bf16))

    inv = 1.0 / THETA ** (np.arange(0, HEAD_DIM, 2, dtype=np.float32) / HEAD_DIM)
    fr = np.outer(inv, np.arange(t_len, dtype=np.float32))  # [64, T]
    cosT = np.cos(fr).astype(np.float32)
    sinT = np.sin(fr).astype(np.float32)
    ccat = np.concatenate([cosT, cosT], axis=0).astype(bf16)   # [128, T]
    scat = np.concatenate([-sinT, sinT], axis=0).astype(bf16)  # [128, T]

    r = np.arange(128)[:, None]
    qc = np.arange(CH)[None, :]
    qh = np.arange(128)[None, :]
    mask = np.concatenate(
        [(r <= qc), (r <= qh)], axis=1
    ).astype(bf16)  # [128, 384]: own kv tile 0 vs all q; own kv tile 1 vs q-half b

    in_maps = []
    for core in range(8):
        b, hg = core // 4, core % 4
        in_maps.append({
            "xt": xts[b], "wq": wq_s[hg], "wk": wk_s[hg], "wv": wv_s[hg],
            "wo": wo_s[hg], "ccat": ccat, "scat": scat, "mask": mask,
        })
    return in_maps


def kernel(x, Wq, Wk, Wv, Wo):
    global _NC, LAST_EXEC_NS
    from concourse.bass_utils import run_bass_kernel_spmd

    profile = bool(os.environ.get("KERNEL_PROFILE"))
    if profile:
        try:
            import hook_util
            hook_util.install()
            hook_util.patch_upload()
        except ImportError:
            profile = False

    in_maps = _prep_inputs(x, Wq, Wk, Wv, Wo)
    if _NC is None:
        _NC = _build_nc()

    kwargs = {}
    if profile:
        kwargs["tmpdir"] = os.environ.get("KERNEL_TRACE_DIR") or None
    res = run_bass_kernel_spmd(
        _NC, in_maps, core_ids=list(range(8)), trace=profile, **kwargs
    )
    LAST_EXEC_NS = res.exec_time_ns

    out = np.zeros((B, T, DM), dtype=np.float32)
    for core in range(8):
        out[core // 4] += res.results[core]["y"]
    return out
